# revision 34
# baseline (speedup 1.0000x reference)
"""Trainium2 Bass kernel for nn_Attention (dense transformer attention layer).

Reference semantics (bug-faithful to the source):
  - Q = x @ wq.T ; V = x @ wv.T ; K-projection is DEAD CODE (the reference
    overwrites xk with the double-angle-rotated Q, so wk never matters).
  - rot = double-angle RoPE applied to Q; keys == rot(Q).
  - start_pos == 0 and t == MAX_SEQ, so the KV cache contents never matter.
  - scores = rotQ @ rotQ.T / sqrt(HD) + mask ; P = softmax ; O = P @ V
  - out = O @ wo.T

Sharding (8 cores): core c -> batch b = c//2, head-half h = c%2 (8 of 16
heads).  Q/V projections + attention are (batch x head-half) parallel; each
core AllGathers its full-T per-head attention output within its pair, then
runs the output projection on its token-half with a full-D contraction.

Schedule (single TileContext, dataflow-overlapped):
  1. V projection for all tokens (x-stationary matmuls), releases wv.
  2. Per token-quarter n: Q^T projection (weight-stationary matmuls produce
     Q^T [feat, tok] directly -- no PE transposes; RoPE runs in transposed
     layout on r|i partition halves, enabled by a host-side deinterleaving
     column permutation of wq) followed by attention chunk n of all heads.
  3. Attention: scores in [k, q] layout (symmetric Gram matrix), exp on the
     Scalar engine over 2-bank PSUM mega-tiles, PV accumulation in PSUM.
     Softmax denominators never touch the PE: exp tiles are accumulated on
     the Vector engine and partition-reduced on GpSimd (partition_all_reduce).
  4. Per-head AllGather (pairs) of full-T rows, then the output projection.

All large inputs are host-cast to bf16 (compute dtype) to halve HBM traffic.
"""

import math
import sys

import numpy as np

sys.path.insert(0, "/opt/trn_rl_repo")

import concourse.bacc as bacc
import concourse.mybir as mybir
from concourse import bass_isa
from concourse.tile import TileContext

F32 = mybir.dt.float32
BF16 = mybir.dt.bfloat16

B = 4
T = 2048
D = 2048
H = 16
HD = 128
N_CORES = 8
PAIRS = [[0, 1], [2, 3], [4, 5], [6, 7]]


def build_nc(T, D, H):
    HD = 128
    assert D == H * HD
    NH = H // 2          # heads per core (8)
    DQ = NH * HD         # own q/v feature count (1024)
    TH = T // 2          # token half
    NT = T // 128        # token tiles (16)
    ND = D // 128        # d tiles (16)
    NQ = 4               # token quarters
    XTQ = T // NQ        # tokens per quarter (512)
    QPC = XTQ // 128     # token tiles per quarter (4)
    CH = XTQ             # attention q-chunk width (512)
    scale = 1.0 / math.sqrt(HD)

    nc = bacc.Bacc(target_bir_lowering=False, num_devices=N_CORES)

    # inputs are host-tiled to match their SBUF images exactly, so every
    # DMA reads long contiguous runs per partition
    xt = nc.declare_dram_parameter("xt", [NQ, 128, ND * XTQ], BF16, isOutput=False)
    wqt = nc.declare_dram_parameter("wqt", [128, ND * DQ], BF16, isOutput=False)
    wvt = nc.declare_dram_parameter("wvt", [128, ND * DQ], BF16, isOutput=False)
    wot = nc.declare_dram_parameter("wot", [NQ, 128, ND * 512], BF16, isOutput=False)
    mkt = nc.declare_dram_parameter("maskt", [128, 128], BF16, isOutput=False)
    # host-precomputed double-angle tables (cols 0:T cos2, T:2T sin2)
    fcs = nc.declare_dram_parameter("fcs", [64, 2 * T], F32, isOutput=False)
    out = nc.declare_dram_parameter("out", [TH, D], F32, isOutput=True)

    # per-(head, chunk) pair exchange of attention output rows (chunk-major
    # so each chunk's collective operates on a contiguous block)
    ag_in = [nc.dram_tensor(f"agi{e}", [4, 128, T // 4], BF16) for e in range(NH)]
    ag_out = [nc.dram_tensor(f"ago{e}", [4, 2, 128, T // 4], BF16) for e in range(NH)]

    with TileContext(nc) as tc:
        import concourse.bass as bass_mod

        pid = nc.partition_id()
        h_idx = pid % 2
        off_own = h_idx * TH         # this core's token-half offset
        peer_i = 1 - h_idx           # peer's index within the pair

        # ---------------- persistent / long-lived pools -----------------
        _cm = {}

        def popen(name, **kw):
            cm = tc.tile_pool(name=name, **kw)
            _cm[name] = cm
            return cm.__enter__()

        def pclose(name):
            _cm.pop(name).__exit__(None, None, None)

        p_c2s2 = popen("c2s2", bufs=1)
        p_rotqt = popen("rotqt", bufs=1)
        p_vsb = popen("vsb", bufs=1)
        p_wqt = popen("wqt", bufs=1, side="right")
        p_xt = popen("xt", bufs=3, side="right")
        p_misc = popen("misc", bufs=1)
        # persistent attention PSUM pools at the stack bottom; the top
        # 4-5 banks alternate between the projection pool (Q quarters)
        # and the 2-bank score mega-tiles (attention chunks)
        p_psO = popen("psO", bufs=2, space="PSUM")
        p_psD = popen("psD", bufs=1, space="PSUM")
        p_proj = popen("projps", bufs=2, space="PSUM")
        p_wvt = popen("wvt", bufs=1, side="right")

        rotqt = p_rotqt.tile([128, NH * T], BF16, tag="rotqt")
        v_sb = p_vsb.tile([128, NT * DQ], BF16, tag="vsb")
        # double-angle tables, both on partitions 0:64 (cols 0:T c2, T:2T s2)
        c2s2 = p_c2s2.tile([64, 2 * T], F32, tag="c2s2")
        mkt_sb = p_misc.tile([128, 128], BF16, tag="mkt")
        ones_sb = p_misc.tile([128, 1], BF16, tag="ones")
        nc.vector.memset(ones_sb[:, :], 1.0)

        # weights: 3D-AP DMAs in dk-quarters, alternating trigger queues so
        # the first V matmuls are gated on only the earliest slices
        QK = ND // 4
        wvt_sb = p_wvt.tile([128, ND * DQ], BF16, tag="wvt")
        wqt_sb = p_wqt.tile([128, ND * DQ], BF16, tag="wqt")

        def load_wslices(dst_sb, src, engs, nsl=4):
            for q4 in range(nsl):
                w = ND * DQ // nsl
                cols = slice(q4 * w, (q4 + 1) * w)
                engs[q4 % len(engs)].dma_start(
                    out=dst_sb[:, cols], in_=src[:, cols]
                )



        def load_xt_quarter(n, engs, nsl=4):
            xt_sb = p_xt.tile([128, ND * XTQ], BF16, tag="xtq")
            for q4 in range(nsl):
                w = ND * XTQ // nsl
                cols = slice(q4 * w, (q4 + 1) * w)
                engs[q4 % len(engs)].dma_start(
                    out=xt_sb[:, cols],
                    in_=xt[n, :, cols],
                )
            return xt_sb

        def prep_rope_tables():
            nc.scalar.dma_start(out=c2s2[:, :], in_=fcs[:, :])

        # ---------------- phase 1: V projection (all tokens) -------------
        xt_pending = [load_xt_quarter(0, [nc.sync, nc.scalar], nsl=8)]
        load_wslices(wvt_sb, wvt, [nc.sync, nc.scalar], nsl=8)
        xt_pending.append(load_xt_quarter(1, [nc.gpsimd]))
        for n in range(NQ):
            xt_sb = xt_pending.pop(0)
            if n < NQ - 1:
                xt_pending.append(
                    load_xt_quarter(
                        n + 2 if n < NQ - 2 else 0,
                        [nc.sync] if n % 2 else [nc.gpsimd],
                    )
                )
            if n == 0:
                nc.gpsimd.dma_start(out=mkt_sb[:, :], in_=mkt[:, :])
            if n == 1:
                # deferred loads: not needed until the Q phase
                load_wslices(wqt_sb, wqt, [nc.scalar, nc.gpsimd])
                prep_rope_tables()
            for j in range(QPC):
                tb = n * QPC + j
                for qc in range(2):
                    ps_v = p_proj.tile([128, 512], F32, tag="ps")
                    for dk in range(ND):
                        nc.tensor.matmul(
                            ps_v[:, :],
                            xt_sb[:, dk * XTQ + j * 128 : dk * XTQ + (j + 1) * 128],
                            wvt_sb[:, dk * DQ + qc * 512 : dk * DQ + (qc + 1) * 512],
                            start=(dk == 0),
                            stop=(dk == ND - 1),
                        )
                    nc.vector.tensor_copy(
                        v_sb[:, tb * DQ + qc * 512 : tb * DQ + (qc + 1) * 512],
                        ps_v[:, :],
                    )
        pclose("wvt")

        # attention-phase pools (SBUF ring space freed by wvt)
        p_wot0 = popen("wot0", bufs=1)
        wot0_sb = p_wot0.tile([128, ND * 512], BF16, tag="wot0")
        p_tt = popen("ttmp", bufs=2)
        p_pt = popen("pt", bufs=6)
        p_rcp = popen("rcp", bufs=2)
        p_otc = popen("otc", bufs=3)
        p_psS = popen("psS", bufs=3, space="PSUM")

        # ---------------- phase 2: Q^T + rope, interleaved attention ------
        def q_head(n, f, xt_sb):
            ps_q = p_proj.tile([128, 512], F32, tag="ps")
            for dk in range(ND):
                nc.tensor.matmul(
                    ps_q[:, :],
                    wqt_sb[:, dk * DQ + f * 128 : dk * DQ + (f + 1) * 128],
                    xt_sb[:, dk * XTQ : (dk + 1) * XTQ],
                    start=(dk == 0),
                    stop=(dk == ND - 1),
                )
            # rope in [feat, tok] layout: rows 0:64 real, 64:128 imag.
            # muls read PSUM+SBUF (mixed spaces, base-partition rule
            # exempt); the final sub/add reads two base-0 SBUF temps.
            qr = ps_q[0:64, :]
            qi = ps_q[64:128, :]
            c2n = c2s2[:, n * XTQ : (n + 1) * XTQ]
            s2n = c2s2[:, T + n * XTQ : T + (n + 1) * XTQ]
            col = slice(f * T + n * XTQ, f * T + (n + 1) * XTQ)
            t1a = p_tt.tile([64, 512], BF16, tag="a")
            t1b = p_tt.tile([64, 512], BF16, tag="b")
            nc.vector.tensor_mul(t1a[:, :], qr, c2n)
            nc.vector.tensor_mul(t1b[:, :], qi, s2n)
            nc.vector.tensor_sub(rotqt[0:64, col], t1a[:, :], t1b[:, :])
            t2a = p_tt.tile([64, 512], BF16, tag="c")
            t2b = p_tt.tile([64, 512], BF16, tag="d")
            nc.vector.tensor_mul(t2a[:, :], qr, s2n)
            nc.vector.tensor_mul(t2b[:, :], qi, c2n)
            nc.vector.tensor_add(rotqt[64:128, col], t2a[:, :], t2b[:, :])

        def emit_ag(eta, c):
            nc.gpsimd.collective_compute(
                "AllGather",
                mybir.AluOpType.bypass,
                replica_groups=PAIRS,
                ins=[ag_in[eta][c : c + 1, :, :].opt()],
                outs=[ag_out[eta][c : c + 1, :, :, :].opt()],
            )

        def attn_head(c, eta):
            KC = (c + 1) * QPC
            q0 = c * CH
            ps_o = p_psO.tile([128, CH], F32, tag="pso")
            ps_d = p_psD.tile([1, CH], F32, tag="psd")
            for kt in range(KC):
                qo = max(0, (kt - c * QPC) * 128)
                ps_s = p_psS.tile([128, CH], F32, tag="pss")
                pt = p_pt.tile([128, CH], BF16, tag="pt")
                nc.tensor.matmul(
                    ps_s[:, qo:CH],
                    rotqt[:, eta * T + kt * 128 : eta * T + kt * 128 + 128],
                    rotqt[:, eta * T + q0 + qo : eta * T + q0 + CH],
                    start=True,
                    stop=True,
                )
                nc.scalar.activation(
                    pt[:, qo:CH],
                    ps_s[:, qo:CH],
                    mybir.ActivationFunctionType.Exp,
                    scale=scale,
                )
                if kt >= c * QPC:  # diagonal block: zero the causal part
                    nc.vector.tensor_mul(
                        pt[:, qo : qo + 128],
                        pt[:, qo : qo + 128],
                        mkt_sb[:, :],
                    )
                nc.tensor.matmul(
                    ps_o[:, qo:CH],
                    v_sb[:, kt * DQ + eta * 128 : kt * DQ + eta * 128 + 128],
                    pt[:, qo:CH],
                    start=(kt == 0),
                    stop=(kt == KC - 1),
                )
                nc.tensor.matmul(
                    ps_d[:, qo:CH],
                    ones_sb[:, :],
                    pt[:, qo:CH],
                    start=(kt == 0),
                    stop=(kt == KC - 1),
                )
            rcp = p_rcp.tile([1, CH], F32, tag="rcp")
            rcpb = p_rcp.tile([128, CH], F32, tag="rcpb")
            nc.vector.reciprocal_approx_fast(rcp[:, :], ps_d[:, :])
            nc.gpsimd.partition_broadcast(rcpb[:, :], rcp[:, :])
            otc = p_otc.tile([128, CH], BF16, tag="otc")
            nc.vector.tensor_mul(otc[:, :], ps_o[:, :], rcpb[:, :])
            nc.sync.dma_start(out=ag_in[eta][c, :, :], in_=otc[:, :])
            # stagger collectives one head behind the broadcasts so the
            # gpsimd queue's wait on each AG never blocks the next PB
            if eta >= 1:
                emit_ag(eta - 1, c)

        # Q(0) stands alone; thereafter Q(n+1) head f interleaves with
        # attention chunk n head f, so independent projection matmuls fill
        # the PE queue wherever attention stalls on the exp pipeline.
        xtq = xt_pending.pop(0)
        for f in range(NH):
            q_head(0, f, xtq)
        nc.scalar.dma_start(out=wot0_sb[:, :], in_=wot[0, :, :])
        for n in range(NQ):
            xt_prev, xtq = xtq, None
            if n + 1 < NQ:
                xtq = load_xt_quarter(
                    n + 1, [nc.gpsimd] if n % 2 else [nc.sync]
                )
            for eta in range(NH):
                attn_head(n, eta)
                if n + 1 < NQ:
                    q_head(n + 1, eta, xtq)
            emit_ag(NH - 1, n)

        # release attention pools and the right-side input pools; the
        # long-lived left pools (c2s2/rotqt/vsb/misc/projps/wot0) stay open
        for name in ("psS", "otc", "rcp", "pt", "ttmp",
                     "projps", "psD", "psO", "xt", "wqt"):
            pclose(name)

        # ---------------- phase 3: gather + output projection -------------
        p_ofull = popen("ofull", bufs=1)
        p_wot = popen("wot", bufs=2, side="right")
        p_osb = popen("osb", bufs=3)
        p_psOut = popen("psOut", bufs=2, space="PSUM")

        # my-half tokens are chunks 2*h_idx and 2*h_idx+1 (dynamic indices)
        o_full = p_ofull.tile([128, 2 * NH * TH], BF16, tag="ofull")
        for j in range(2):
            for r16 in range(2 * NH):
                eta = r16 % NH
                ck = 2 * h_idx + j
                if r16 < NH:
                    # own rows come straight from our staging (no AG dep)
                    src_ap = ag_in[eta][bass_mod.ds(ck, 1), :, :]
                else:
                    src_ap = ag_out[eta][
                        bass_mod.ds(ck, 1), bass_mod.ds(peer_i, 1), :, :
                    ]
                nc.sync.dma_start(
                    out=o_full[:, r16 * TH + j * CH : r16 * TH + (j + 1) * CH],
                    in_=src_ap,
                )

        NDO = D // 512
        for do in range(NDO):
            if do == 0:
                wot_sb = wot0_sb
            else:
                wot_sb = p_wot.tile([128, ND * 512], BF16, tag="wot")
                nc.sync.dma_start(out=wot_sb[:, :], in_=wot[do, :, :])
            for tb8 in range(TH // 128):
                ps_out = p_psOut.tile([128, 512], F32, tag="psout")
                for r16 in range(2 * NH):
                    nc.tensor.matmul(
                        ps_out[:, :],
                        o_full[:, r16 * TH + tb8 * 128 : r16 * TH + tb8 * 128 + 128],
                        wot_sb[:, r16 * 512 : (r16 + 1) * 512],
                        start=(r16 == 0),
                        stop=(r16 == 2 * NH - 1),
                    )
                osb = p_osb.tile([128, 512], F32, tag="osb")
                nc.vector.tensor_copy(osb[:, :], ps_out[:, :])
                nc.sync.dma_start(
                    out=out[tb8 * 128 : (tb8 + 1) * 128, do * 512 : (do + 1) * 512],
                    in_=osb[:, :],
                )

        for name in reversed(list(_cm)):
            pclose(name)

    nc.finalize()
    return nc


def host_prep(T, D, H, x, wq, wv, wo, mask, freqs_cos, freqs_sin):
    """Build per-core in_maps (host-side layout/dtype prep only)."""
    import ml_dtypes

    bf16 = ml_dtypes.bfloat16
    HD = 128
    NH = H // 2
    DQ = NH * HD
    # 0/1 upper-triangle keep-mask for the post-exp diagonal-block zeroing
    mkt = np.ascontiguousarray(np.triu(np.ones((128, 128), np.float32)).astype(bf16))
    fcn = np.asarray(freqs_cos, np.float32)  # [T, 64]
    fsn = np.asarray(freqs_sin, np.float32)
    c2t = fcn.T * fcn.T - fsn.T * fsn.T   # cos(2a)  [64, T]
    s2t = 2.0 * fcn.T * fsn.T             # sin(2a)
    fcs = np.ascontiguousarray(np.concatenate([c2t, s2t], axis=1).astype(np.float32))
    # deinterleave permutation: within each head block, (r0,r1,..,i0,i1,..)
    perm = np.concatenate(
        [hb * 128 + np.r_[0:128:2, 1:128:2] for hb in range(NH)]
    )
    wot_full = np.asarray(wo, np.float32).T  # [din2, dout]
    in_maps = []
    for c in range(N_CORES):
        b, h = c // 2, c % 2
        rows = slice(h * DQ, (h + 1) * DQ)
        wqt_c = np.asarray(wq[rows], np.float32).T[:, perm]
        # o_full rows are in local head order (own heads first): permute wot
        wot_c = np.concatenate(
            [wot_full[h * DQ : (h + 1) * DQ], wot_full[(1 - h) * DQ : (2 - h) * DQ]],
            axis=0,
        )
        # tile to SBUF images: [dk-major columns, partition-major rows]
        def timg(a, ncol):  # [D, C] -> [C//ncol, 128, 16*ncol]
            ND_, C = a.shape[0] // 128, a.shape[1]
            return np.ascontiguousarray(
                a.reshape(ND_, 128, C // ncol, ncol)
                .transpose(2, 1, 0, 3)
                .reshape(C // ncol, 128, ND_ * ncol)
                .astype(bf16)
            )
        xtc = timg(np.asarray(x[b], np.float32).T, 512)          # [4,128,8192]
        wqtc = timg(wqt_c, DQ).reshape(128, -1)                  # [128,16384]
        wvtc = timg(np.asarray(wv[rows], np.float32).T, DQ).reshape(128, -1)
        wotc = timg(wot_c, 512)                                  # [4,128,8192]
        in_maps.append(
            {
                "xt": xtc,
                "wqt": wqtc,
                "wvt": wvtc,
                "wot": wotc,
                "maskt": mkt,
                "fcs": fcs,
            }
        )
    return in_maps


_NC_CACHE = {}


def run(T, D, H, inputs, trace=False):
    from concourse.bass_utils import run_bass_kernel_spmd

    key = (T, D, H)
    if key not in _NC_CACHE:
        _NC_CACHE[key] = build_nc(T, D, H)
    nc = _NC_CACHE[key]
    in_maps = host_prep(
        T, D, H,
        inputs["x"], inputs["wq"], inputs["wv"], inputs["wo"],
        inputs["mask"], inputs["freqs_cos"], inputs["freqs_sin"],
    )
    res = run_bass_kernel_spmd(nc, in_maps, core_ids=list(range(N_CORES)), trace=trace)
    B_ = np.asarray(inputs["x"]).shape[0]
    TH = T // 2
    out = np.empty((B_, T, D), np.float32)
    for c in range(N_CORES):
        b, h = c // 2, c % 2
        out[b, h * TH : (h + 1) * TH, :] = res.results[c]["out"]
    return out, res


def kernel(**inputs):
    out, _ = run(T, D, H, inputs, trace=False)
    return out
